# revision 13
# baseline (speedup 1.0000x reference)
"""Trainium2 Bass kernel for nn_EntityNamePredictor (BIO span extraction +
sum pooling + entity-type projection).

Strategy (pure data-parallel over batch, 8 samples per core):
  - Host: label preprocessing only (cumsum of B-tags -> per-token local slot
    ids relative to a fixed 256-token chunk grid), plus a transpose+cast of
    hidden_layers to [b, d, t] layout so the on-device projection matmul can
    contract over d with natural APs.
  - Device (per core, uniform SPMD program):
      proj^T[32, t] = E^T @ hiddenT          (PE, E stationary)
      proj[t, 32]   = PE transpose of proj^T
      onehot[t, l]  = (iota == seg_rel)      (DVE, exact 0/1 in bf16)
      chunk_logits[l, 32] += onehot^T @ proj (PE, per 256-token chunk)
    Each 256-token chunk emits a [128, 32] partial-logit block at a fixed
    DRAM address (slot placement is data-dependent, so it is resolved on
    host during unshard).
  - Host: overlap-add the per-chunk blocks into the final [B, S, 32] logits.

The segment-sum is exact: the one-hot matrix is 0/1 (exact in bf16) and all
accumulation happens in fp32 PSUM.  The only precision loss is the rounding
of hidden_layers / entity_type_embs entering the projection matmul:
  mode "bf16":  hidT+emb in bf16 (half DMA)       ~5e-3 rel
  mode "f32r":  hidT+emb in float32r              hw-dependent
  mode "split": hidT as hi+lo bf16, emb f32r      ~1e-5 rel
"""

import json

import numpy as np
import ml_dtypes

import concourse.bass as bass
import concourse.bass2jax as bass2jax
import concourse.bass_utils as bass_utils
import concourse.mybir as mybir
import concourse.tile as tile
from concourse.bass_utils import run_bass_kernel_spmd
from concourse.vector_clock import ScopedClock
import bass_rust

# ----------------------------------------------------------------------------
# Problem constants (hardcoded per the harness contract)
BSZ, SEQ, DIM, NENT = 64, 4096, 400, 32
NCORES = 8
BPC = BSZ // NCORES            # samples per core
KT = 128                       # tokens per k-tile
NKT = SEQ // KT                # k-tiles per sample (32)
CH = 256                       # tokens per chunk
NCH = SEQ // CH                # chunks per sample (16)
GRP = 512                      # tokens per projection group
NGRP = SEQ // GRP              # groups per sample (8)
W = 128                        # slot window per chunk (local slot ids 0..127)
NDSL = 4                       # d-slices of 100

BF16 = mybir.dt.bfloat16
F32 = mybir.dt.float32
F32R = mybir.dt.float32r
NP_BF16 = np.dtype(ml_dtypes.bfloat16)

# precision mode: "bf16", "f32r", "split"
MODE = "bf16"


def _patch_tile_tail_drain():
    """This walrus build rejects instructions with >2 sem waits; the Tile
    tail drain aggregates one wait per live semaphore.  Re-distribute the
    waits across single-wait SP no-ops emitted just before the drain."""
    if getattr(tile.TileContext, "_tail_drain_patched", False):
        return

    def _drain_and_barrier(self, tick_clock, wait_clock):
        nc = self.nc
        probe = nc.sync.nop(nofuse=True, hint="tail_drain_waits")
        wait_clock.add_sem_waits(
            probe.ins, ScopedClock({None: tick_clock.global_clock})
        )
        si = probe.ins.sync_info
        waits = list(si.on_wait) if si is not None else []
        upds = list(si.on_update) if si is not None else []
        probe.ins.sync_info = bass_rust.SyncInfo(on_wait=waits[:1], on_update=upds)
        for w in waits[1:]:
            n = nc.sync.nop(nofuse=True, hint="tail_drain_waits")
            n.ins.sync_info = bass_rust.SyncInfo(on_wait=[w], on_update=[])
        nc.sync.drain()
        nc.all_engine_barrier()
        assert self.sems is not None
        popped = nc._tile_sem_poison_stack.pop()
        assert popped is self._sem_poison
        nc.clear_and_free_semaphores(list(self.sems.allocated().values()))
        nc.all_engine_barrier()

    tile.TileContext._drain_and_barrier = _drain_and_barrier
    tile.TileContext._tail_drain_patched = True


# Max sync waits one instruction may carry through this walrus build.
_MAXW = 1


def _split_excess_waits(bir_json: bytes) -> bytes:
    """Walrus rejects instructions with too many sync-wait commands.  Move
    excess waits onto pure-wait EventSemaphore instructions inserted just
    before the offending instruction on the same engine (the sequencer
    executes its stream in order, so this is semantically identical)."""
    d = json.loads(bir_json)
    uid = 0
    changed = False
    for fn in d.get("functions", []):
        for blk in fn.get("blocks", []):
            out = []
            for inst in blk.get("instructions", []):
                si = inst.get("sync_info")
                ow = (si or {}).get("on_wait") or []
                if len(ow) > _MAXW:
                    changed = True
                    extra, keep = ow[:-_MAXW], ow[-_MAXW:]
                    for i in range(0, len(extra), _MAXW):
                        out.append({
                            "debug": 0,
                            "engine": inst["engine"],
                            "ins": [],
                            "outs": [],
                            "name": f"{inst['name']}-xw{uid}",
                            "opcode": "EventSemaphore",
                            "sync_info": {
                                "on_update": [],
                                "on_wait": extra[i : i + _MAXW],
                            },
                        })
                        uid += 1
                    si["on_wait"] = keep
                out.append(inst)
            blk["instructions"] = out
    if not changed:
        return bir_json
    return json.dumps(d).encode()


def _patch_compile_hook():
    if getattr(bass_utils, "_split_waits_patched", False):
        return
    orig = bass_utils.compile_bir_kernel

    def patched(bir_json, tmpdir, neff_name="file.neff"):
        return orig(_split_excess_waits(bir_json), tmpdir, neff_name=neff_name)

    bass_utils.compile_bir_kernel = patched
    if getattr(bass2jax, "compile_bir_kernel", None) is orig:
        bass2jax.compile_bir_kernel = patched
    bass_utils._split_waits_patched = True


def _build_program(mode: str, bufs_hid: int = 2, loop_k: int = 0):
    """loop_k > 0 wraps the whole body in a For_i that repeats it loop_k
    times (benchmarking only — amortizes host<->device transfer)."""
    nc = bass.Bass("TRN2", target_bir_lowering=False, debug=False)

    hid_dt = {"bf16": BF16, "f32r": F32R, "split": BF16}[mode]
    emb_dt = {"bf16": BF16, "f32r": F32R, "split": F32R}[mode]
    prj_dt = BF16 if mode == "bf16" else F32  # projT sbuf / transpose dtype
    split_proj = mode in ("f32r", "split")    # hi/lo split of proj for mm_seg

    # DRAM parameters
    hidT = nc.dram_tensor("hidT", [BPC, DIM, SEQ], hid_dt, kind="ExternalInput")
    if mode == "split":
        hidT_lo = nc.dram_tensor(
            "hidT_lo", [BPC, DIM, SEQ], BF16, kind="ExternalInput"
        )
    segrel = nc.dram_tensor("segrel", [BPC, KT, NKT], F32, kind="ExternalInput")
    iota_in = nc.dram_tensor("iota", [KT, W], F32, kind="ExternalInput")
    emb = nc.dram_tensor("emb", [100, NDSL, NENT], emb_dt, kind="ExternalInput")
    ident = nc.dram_tensor("ident", [NENT, NENT], prj_dt, kind="ExternalInput")
    out = nc.dram_tensor(
        "chunk_out", [BPC, NCH, W, NENT], F32, kind="ExternalOutput"
    )

    with tile.TileContext(nc) as tc:
        with (
            tc.tile_pool(name="const", bufs=1) as cpool,
            tc.tile_pool(name="hid", bufs=bufs_hid) as hpool,
            tc.tile_pool(name="proj", bufs=3) as ppool,
            tc.tile_pool(name="oneh", bufs=3) as opool,
            tc.tile_pool(name="outp", bufs=2) as outpool,
            tc.tile_pool(name="ps_projT", bufs=2, space="PSUM") as ps_projT,
            tc.tile_pool(name="ps_tp", bufs=2, space="PSUM") as ps_tp,
            tc.tile_pool(name="ps_log", bufs=3, space="PSUM") as ps_log,
        ):
            iota_t = cpool.tile([KT, W], F32)
            nc.sync.dma_start(iota_t[:], iota_in[:])
            emb_t = cpool.tile([100, NDSL, NENT], emb_dt)
            nc.sync.dma_start(emb_t[:], emb[:])
            ident_t = cpool.tile([NENT, NENT], prj_dt)
            nc.sync.dma_start(ident_t[:], ident[:])
            # all samples' seg_rel columns in one load: [t, (b, k)]
            sr_t = cpool.tile([KT, BPC, NKT], F32)
            nc.sync.dma_start(
                sr_t[:], segrel.rearrange("b t k -> t b k")
            )

            import contextlib
            loop_cm = tc.For_i(0, loop_k, 1) if loop_k else (
                contextlib.nullcontext()
            )
            with loop_cm:
              for b in range(BPC):
                # one whole-sample load per (pass): [100, 4, 4096]
                ht = hpool.tile([100, NDSL, SEQ], hid_dt, tag="hid")
                nc.sync.dma_start(
                    ht[:], hidT[b].rearrange("(j p) t -> p j t", p=100)
                )
                if mode == "split":
                    ht_lo = hpool.tile([100, NDSL, SEQ], BF16, tag="hidlo")
                    nc.sync.dma_start(
                        ht_lo[:], hidT_lo[b].rearrange("(j p) t -> p j t", p=100)
                    )
                # per-sample output accumulator in SBUF, one DMA at the end
                ob = outpool.tile([W, NCH, NENT], F32, tag="outsb")

                for g in range(NGRP):
                    # ---- projection: projT[32, 512] = E^T @ hiddenT group
                    pT = ps_projT.tile([NENT, GRP], F32, tag="pT")
                    n_pass = 2 if mode == "split" else 1
                    mm = 0
                    for ip in range(n_pass):
                        src_h = ht if ip == 0 else ht_lo
                        for j in range(NDSL):
                            nc.tensor.matmul(
                                pT[:],
                                emb_t[:, j, :],
                                src_h[:, j, g * GRP : (g + 1) * GRP],
                                start=(mm == 0),
                                stop=(mm == NDSL * n_pass - 1),
                            )
                            mm += 1

                    # cast / copy projT to SBUF in transpose dtype
                    pT_sb = ppool.tile([NENT, GRP], prj_dt, tag="pTsb")
                    nc.scalar.copy(pT_sb[:], pT[:])

                    # ---- transpose projT -> proj [128 tok, 32] x4 slices
                    tp = ps_tp.tile([KT, 4 * NENT], prj_dt, tag="tp")
                    for j in range(4):
                        nc.tensor.matmul(
                            tp[:, j * NENT : (j + 1) * NENT],
                            pT_sb[:, j * KT : (j + 1) * KT],
                            ident_t[:],
                            is_transpose=True,
                        )
                    proj_sb = ppool.tile([KT, 4 * NENT], BF16, tag="proj")
                    nc.scalar.copy(proj_sb[:], tp[:])
                    if split_proj:
                        proj_lo = ppool.tile([KT, 4 * NENT], BF16, tag="projlo")
                        nc.vector.tensor_sub(proj_lo[:], tp[:], proj_sb[:])

                    # ---- segment-sum: 2 chunks per group
                    for cc in range(2):
                        c = 2 * g + cc
                        pl = ps_log.tile([W, NENT], F32, tag="plog")
                        n_seg = 2 if split_proj else 1
                        mm2 = 0
                        for jj in range(2):  # k-tiles in chunk
                            k = 4 * g + 2 * cc + jj
                            oh = opool.tile([KT, W], BF16, tag="oneh")
                            nc.vector.tensor_scalar(
                                oh[:], iota_t[:], sr_t[:, b, k : k + 1], None,
                                mybir.AluOpType.is_equal,
                            )
                            sl = slice(
                                (2 * cc + jj) * NENT, (2 * cc + jj + 1) * NENT
                            )
                            for ip in range(n_seg):
                                src_p = proj_sb if ip == 0 else proj_lo
                                nc.tensor.matmul(
                                    pl[:],
                                    oh[:],
                                    src_p[:, sl],
                                    start=(mm2 == 0),
                                    stop=(mm2 == 2 * n_seg - 1),
                                )
                                mm2 += 1
                        nc.vector.tensor_copy(ob[:, c, :], pl[:])
                nc.sync.dma_start(
                    out[b].rearrange("c l e -> l c e"), ob[:]
                )
    return nc


def _host_prep(hidden_layers, binary_labels, entity_type_embs, mode):
    hid = np.asarray(hidden_layers, dtype=np.float32)
    lab = np.asarray(binary_labels)
    E = np.asarray(entity_type_embs, dtype=np.float32)

    is_b = (lab == 1)
    seg = np.cumsum(is_b.astype(np.int64), axis=1)  # (B, S) 1-based span count
    valid = ((lab == 1) | (lab == 2)) & (seg > 0)
    num_slots = seg[:, -1].astype(np.int32)

    # chunk grid: base slot of chunk c = open slot at token 256c
    base_c = np.maximum(
        np.concatenate(
            [np.zeros((BSZ, 1), np.int64), seg[:, CH - 1 : -1 : CH] - 1], axis=1
        ),
        0,
    )  # (B, NCH)
    slot = seg - 1  # 0-based slot id of each token (valid tokens only)
    rel = slot - np.repeat(base_c, CH, axis=1)
    if valid.any():
        assert rel[valid].min() >= 0
        assert rel[valid].max() < W, f"chunk width overflow: {rel[valid].max()}"
    segrel = np.where(valid, rel, -1).astype(np.float32)  # (B, S)

    # [b, t_within_ktile, ktile]
    segrelT = np.ascontiguousarray(
        segrel.reshape(BSZ, NKT, KT).transpose(0, 2, 1)
    )

    hidT = np.ascontiguousarray(hid.transpose(0, 2, 1))  # (B, D, S)
    emb4 = np.ascontiguousarray(
        E.reshape(NDSL, 100, NENT).transpose(1, 0, 2)
    )  # [100, 4, 32]
    prep = {"segrelT": segrelT, "base_c": base_c, "num_slots": num_slots}
    if mode == "bf16":
        prep["hidT"] = hidT.astype(NP_BF16)
        prep["emb"] = emb4.astype(NP_BF16)
        prep["ident"] = np.eye(NENT, dtype=np.float32).astype(NP_BF16)
    elif mode == "f32r":
        prep["hidT"] = hidT
        prep["emb"] = emb4
        prep["ident"] = np.eye(NENT, dtype=np.float32)
    else:  # split
        hi = hidT.astype(NP_BF16)
        prep["hidT"] = hi
        prep["hidT_lo"] = (hidT - hi.astype(np.float32)).astype(NP_BF16)
        prep["emb"] = emb4
        prep["ident"] = np.eye(NENT, dtype=np.float32)

    prep["iota"] = np.broadcast_to(
        np.arange(W, dtype=np.float32), (KT, W)
    ).copy()
    return prep


_PROG_CACHE = {}


def kernel(hidden_layers, binary_labels, entity_type_embs, _mode=None,
           _trace=False, _result_box=None):
    mode = _mode or MODE
    _patch_tile_tail_drain()
    _patch_compile_hook()
    prep = _host_prep(hidden_layers, binary_labels, entity_type_embs, mode)

    if mode not in _PROG_CACHE:
        _PROG_CACHE[mode] = _build_program(mode)
    nc = _PROG_CACHE[mode]

    in_maps = []
    for core in range(NCORES):
        sl = slice(core * BPC, (core + 1) * BPC)
        m = {
            "hidT": prep["hidT"][sl],
            "segrel": prep["segrelT"][sl],
            "iota": prep["iota"],
            "emb": prep["emb"],
            "ident": prep["ident"],
        }
        if mode == "split":
            m["hidT_lo"] = prep["hidT_lo"][sl]
        in_maps.append(m)

    res = run_bass_kernel_spmd(
        nc, in_maps, core_ids=list(range(NCORES)), trace=_trace
    )
    if _result_box is not None:
        _result_box.append(res)

    logits = np.zeros((BSZ, SEQ, NENT), dtype=np.float32)
    base_c = prep["base_c"]
    for core in range(NCORES):
        co = res.results[core]["chunk_out"]  # (BPC, NCH, W, NENT)
        for bb in range(BPC):
            b = core * BPC + bb
            for c in range(NCH):
                base = int(base_c[b, c])
                logits[b, base : base + W] += co[bb, c]
    return logits, prep["num_slots"]


# revision 19
# speedup vs baseline: 2.0280x; 2.0280x over previous
"""Trainium2 Bass kernel for nn_EntityNamePredictor (BIO span extraction +
sum pooling + entity-type projection).

Strategy (pure data-parallel over batch, 8 samples per core):
  - Host: label preprocessing only (cumsum of B-tags -> per-token local slot
    ids relative to a fixed 256-token chunk grid), plus a transpose+cast of
    hidden_layers to [b, d, t] layout so the on-device projection matmul can
    contract over d with natural APs.
  - Device (per core, uniform SPMD program):
      proj^T[32, t] = E^T @ hiddenT          (PE, E stationary)
      proj[t, 32]   = PE transpose of proj^T
      onehot[t, l]  = (iota == seg_rel)      (DVE, exact 0/1 in bf16)
      chunk_logits[l, 32] += onehot^T @ proj (PE, per 256-token chunk)
    Each 256-token chunk emits a [128, 32] partial-logit block at a fixed
    DRAM address (slot placement is data-dependent, so it is resolved on
    host during unshard).
  - Host: overlap-add the per-chunk blocks into the final [B, S, 32] logits.

The segment-sum is exact: the one-hot matrix is 0/1 (exact in bf16) and all
accumulation happens in fp32 PSUM.  The only precision loss is the rounding
of hidden_layers / entity_type_embs entering the projection matmul:
  mode "bf16":  hidT+emb in bf16 (half DMA)       ~5e-3 rel
  mode "f32r":  hidT+emb in float32r              hw-dependent
  mode "split": hidT as hi+lo bf16, emb f32r      ~1e-5 rel
"""

import json

import numpy as np
import ml_dtypes

import concourse.bass as bass
import concourse.bass2jax as bass2jax
import concourse.bass_utils as bass_utils
import concourse.mybir as mybir
import concourse.tile as tile
from concourse.bass_utils import run_bass_kernel_spmd
from concourse.vector_clock import ScopedClock
import bass_rust

# ----------------------------------------------------------------------------
# Problem constants (hardcoded per the harness contract)
BSZ, SEQ, DIM, NENT = 64, 4096, 400, 32
NCORES = 8
BPC = BSZ // NCORES            # samples per core
KT = 128                       # tokens per k-tile
NKT = SEQ // KT                # k-tiles per sample (32)
CH = 256                       # tokens per chunk
NCH = SEQ // CH                # chunks per sample (16)
GRP = 512                      # tokens per projection group
NGRP = SEQ // GRP              # groups per sample (8)
W = 128                        # slot window per chunk (local slot ids 0..127)
NDSL = 4                       # d-slices of 100

BF16 = mybir.dt.bfloat16
F32 = mybir.dt.float32
F32R = mybir.dt.float32r
NP_BF16 = np.dtype(ml_dtypes.bfloat16)

# precision mode: "bf16", "f32r", "split"
MODE = "bf16"


def _patch_tile_tail_drain():
    """This walrus build rejects instructions with >2 sem waits; the Tile
    tail drain aggregates one wait per live semaphore.  Re-distribute the
    waits across single-wait SP no-ops emitted just before the drain."""
    if getattr(tile.TileContext, "_tail_drain_patched", False):
        return

    def _drain_and_barrier(self, tick_clock, wait_clock):
        nc = self.nc
        probe = nc.sync.nop(nofuse=True, hint="tail_drain_waits")
        wait_clock.add_sem_waits(
            probe.ins, ScopedClock({None: tick_clock.global_clock})
        )
        si = probe.ins.sync_info
        waits = list(si.on_wait) if si is not None else []
        upds = list(si.on_update) if si is not None else []
        probe.ins.sync_info = bass_rust.SyncInfo(on_wait=waits[:1], on_update=upds)
        for w in waits[1:]:
            n = nc.sync.nop(nofuse=True, hint="tail_drain_waits")
            n.ins.sync_info = bass_rust.SyncInfo(on_wait=[w], on_update=[])
        nc.sync.drain()
        nc.all_engine_barrier()
        assert self.sems is not None
        popped = nc._tile_sem_poison_stack.pop()
        assert popped is self._sem_poison
        nc.clear_and_free_semaphores(list(self.sems.allocated().values()))
        nc.all_engine_barrier()

    tile.TileContext._drain_and_barrier = _drain_and_barrier
    tile.TileContext._tail_drain_patched = True


# Max sync waits one instruction may carry through this walrus build.
_MAXW = 1


def _split_excess_waits(bir_json: bytes) -> bytes:
    """Walrus rejects instructions with too many sync-wait commands.  Move
    excess waits onto pure-wait EventSemaphore instructions inserted just
    before the offending instruction on the same engine (the sequencer
    executes its stream in order, so this is semantically identical)."""
    d = json.loads(bir_json)
    uid = 0
    changed = False
    for fn in d.get("functions", []):
        for blk in fn.get("blocks", []):
            out = []
            for inst in blk.get("instructions", []):
                si = inst.get("sync_info")
                ow = (si or {}).get("on_wait") or []
                if len(ow) > _MAXW:
                    changed = True
                    extra, keep = ow[:-_MAXW], ow[-_MAXW:]
                    for i in range(0, len(extra), _MAXW):
                        out.append({
                            "debug": 0,
                            "engine": inst["engine"],
                            "ins": [],
                            "outs": [],
                            "name": f"{inst['name']}-xw{uid}",
                            "opcode": "EventSemaphore",
                            "sync_info": {
                                "on_update": [],
                                "on_wait": extra[i : i + _MAXW],
                            },
                        })
                        uid += 1
                    si["on_wait"] = keep
                out.append(inst)
            blk["instructions"] = out
    if not changed:
        return bir_json
    return json.dumps(d).encode()


def _patch_compile_hook():
    if getattr(bass_utils, "_split_waits_patched", False):
        return
    orig = bass_utils.compile_bir_kernel

    def patched(bir_json, tmpdir, neff_name="file.neff"):
        return orig(_split_excess_waits(bir_json), tmpdir, neff_name=neff_name)

    bass_utils.compile_bir_kernel = patched
    if getattr(bass2jax, "compile_bir_kernel", None) is orig:
        bass2jax.compile_bir_kernel = patched
    bass_utils._split_waits_patched = True


def _build_program(mode: str, bufs_hid: int = 2, loop_k: int = 0):
    """loop_k > 0 wraps the whole body in a For_i that repeats it loop_k
    times (benchmarking only — amortizes host<->device transfer)."""
    nc = bass.Bass("TRN2", target_bir_lowering=False, debug=False)

    hid_dt = {"bf16": BF16, "f32r": F32R, "split": BF16}[mode]
    emb_dt = {"bf16": BF16, "f32r": F32R, "split": F32R}[mode]
    prj_dt = BF16 if mode == "bf16" else F32  # projT sbuf / transpose dtype
    split_proj = mode in ("f32r", "split")    # hi/lo split of proj for mm_seg

    # DRAM parameters
    hidT = nc.dram_tensor("hidT", [BPC, DIM, SEQ], hid_dt, kind="ExternalInput")
    if mode == "split":
        hidT_lo = nc.dram_tensor(
            "hidT_lo", [BPC, DIM, SEQ], BF16, kind="ExternalInput"
        )
    segrel = nc.dram_tensor("segrel", [BPC, KT, NKT], F32, kind="ExternalInput")
    iota_in = nc.dram_tensor("iota", [KT, W], F32, kind="ExternalInput")
    emb_a_in = nc.dram_tensor("emb_a", [128, 3, NENT], emb_dt,
                              kind="ExternalInput")
    emb_b_in = nc.dram_tensor("emb_b", [16, NENT], emb_dt,
                              kind="ExternalInput")
    ident = nc.dram_tensor("ident", [NENT, NENT], prj_dt, kind="ExternalInput")
    out = nc.dram_tensor(
        "chunk_out", [BPC, NCH, W, NENT], F32, kind="ExternalOutput"
    )

    with tile.TileContext(nc) as tc:
        with (
            tc.tile_pool(name="const", bufs=1) as cpool,
            tc.tile_pool(name="hid", bufs=bufs_hid) as hpool,
            tc.tile_pool(name="proj", bufs=3) as ppool,
            tc.tile_pool(name="oneh", bufs=3) as opool,
            tc.tile_pool(name="outp", bufs=2) as outpool,
            tc.tile_pool(name="ps_projT", bufs=2, space="PSUM") as ps_projT,
            tc.tile_pool(name="ps_tp", bufs=2, space="PSUM") as ps_tp,
            tc.tile_pool(name="ps_log", bufs=3, space="PSUM") as ps_log,
        ):
            iota_t = cpool.tile([KT, W], F32)
            nc.sync.dma_start(iota_t[:], iota_in[:])
            emb_a = cpool.tile([128, 3, NENT], emb_dt)
            nc.sync.dma_start(emb_a[:], emb_a_in[:])
            emb_b = cpool.tile([16, NENT], emb_dt)
            nc.sync.dma_start(emb_b[:], emb_b_in[:])
            ident_t = cpool.tile([NENT, NENT], prj_dt)
            nc.sync.dma_start(ident_t[:], ident[:])
            # all samples' seg_rel columns in one load: [t, (b, k)]
            sr_t = cpool.tile([KT, BPC, NKT], F32)
            nc.sync.dma_start(
                sr_t[:], segrel.rearrange("b t k -> t b k")
            )

            import contextlib
            loop_cm = tc.For_i(0, loop_k, 1) if loop_k else (
                contextlib.nullcontext()
            )
            with loop_cm:
              for b in range(BPC):
                # whole-sample loads, one DMA per 128-partition d-slice,
                # alternating the two HWDGE engines to spread queues
                dma_engs = [nc.sync, nc.scalar]
                ht = hpool.tile([128, 3, SEQ], hid_dt, tag="hid")
                for j in range(3):
                    dma_engs[j % 2].dma_start(
                        ht[:, j, :], hidT[b, 128 * j : 128 * (j + 1), :]
                    )
                ht_b = hpool.tile([16, SEQ], hid_dt, tag="hidb")
                dma_engs[1].dma_start(ht_b[:], hidT[b, 384:400, :])
                if mode == "split":
                    ht_lo = hpool.tile([128, 3, SEQ], BF16, tag="hidlo")
                    for j in range(3):
                        dma_engs[j % 2].dma_start(
                            ht_lo[:, j, :],
                            hidT_lo[b, 128 * j : 128 * (j + 1), :],
                        )
                    ht_lo_b = hpool.tile([16, SEQ], BF16, tag="hidlob")
                    dma_engs[0].dma_start(ht_lo_b[:], hidT_lo[b, 384:400, :])
                # per-sample output accumulator in SBUF, one DMA at the end
                ob = outpool.tile([W, NCH, NENT], F32, tag="outsb")

                for g in range(NGRP):
                    # ---- projection: projT[32, 512] = E^T @ hiddenT group
                    pT = ps_projT.tile([NENT, GRP], F32, tag="pT")
                    n_pass = 2 if mode == "split" else 1
                    mm = 0
                    gsl = slice(g * GRP, (g + 1) * GRP)
                    for ip in range(n_pass):
                        src3 = ht if ip == 0 else ht_lo
                        src1 = ht_b if ip == 0 else ht_lo_b
                        for j in range(3):
                            nc.tensor.matmul(
                                pT[:],
                                emb_a[:, j, :],
                                src3[:, j, gsl],
                                start=(mm == 0),
                                stop=False,
                            )
                            mm += 1
                        nc.tensor.matmul(
                            pT[:],
                            emb_b[:],
                            src1[:, gsl],
                            start=False,
                            stop=(ip == n_pass - 1),
                        )
                        mm += 1

                    # cast / copy projT to SBUF in transpose dtype
                    pT_sb = ppool.tile([NENT, GRP], prj_dt, tag="pTsb")
                    nc.scalar.copy(pT_sb[:], pT[:])

                    # ---- transpose projT -> proj [128 tok, 32] x4 slices
                    tp = ps_tp.tile([KT, 4 * NENT], prj_dt, tag="tp")
                    for j in range(4):
                        nc.tensor.matmul(
                            tp[:, j * NENT : (j + 1) * NENT],
                            pT_sb[:, j * KT : (j + 1) * KT],
                            ident_t[:],
                            is_transpose=True,
                        )
                    proj_sb = ppool.tile([KT, 4 * NENT], BF16, tag="proj")
                    nc.scalar.copy(proj_sb[:], tp[:])
                    if split_proj:
                        proj_lo = ppool.tile([KT, 4 * NENT], BF16, tag="projlo")
                        nc.vector.tensor_sub(proj_lo[:], tp[:], proj_sb[:])

                    # ---- segment-sum: 2 chunks per group
                    for cc in range(2):
                        c = 2 * g + cc
                        pl = ps_log.tile([W, NENT], F32, tag="plog")
                        n_seg = 2 if split_proj else 1
                        mm2 = 0
                        for jj in range(2):  # k-tiles in chunk
                            k = 4 * g + 2 * cc + jj
                            oh = opool.tile([KT, W], BF16, tag="oneh")
                            nc.vector.tensor_scalar(
                                oh[:], iota_t[:], sr_t[:, b, k : k + 1], None,
                                mybir.AluOpType.is_equal,
                            )
                            sl = slice(
                                (2 * cc + jj) * NENT, (2 * cc + jj + 1) * NENT
                            )
                            for ip in range(n_seg):
                                src_p = proj_sb if ip == 0 else proj_lo
                                nc.tensor.matmul(
                                    pl[:],
                                    oh[:],
                                    src_p[:, sl],
                                    start=(mm2 == 0),
                                    stop=(mm2 == 2 * n_seg - 1),
                                )
                                mm2 += 1
                        nc.vector.tensor_copy(ob[:, c, :], pl[:])
                nc.sync.dma_start(
                    out[b].rearrange("c l e -> l c e"), ob[:]
                )
    return nc


def _host_prep(hidden_layers, binary_labels, entity_type_embs, mode):
    hid = np.asarray(hidden_layers, dtype=np.float32)
    lab = np.asarray(binary_labels)
    E = np.asarray(entity_type_embs, dtype=np.float32)

    is_b = (lab == 1)
    seg = np.cumsum(is_b.astype(np.int64), axis=1)  # (B, S) 1-based span count
    valid = ((lab == 1) | (lab == 2)) & (seg > 0)
    num_slots = seg[:, -1].astype(np.int32)

    # chunk grid: base slot of chunk c = open slot at token 256c
    base_c = np.maximum(
        np.concatenate(
            [np.zeros((BSZ, 1), np.int64), seg[:, CH - 1 : -1 : CH] - 1], axis=1
        ),
        0,
    )  # (B, NCH)
    slot = seg - 1  # 0-based slot id of each token (valid tokens only)
    rel = slot - np.repeat(base_c, CH, axis=1)
    if valid.any():
        assert rel[valid].min() >= 0
        assert rel[valid].max() < W, f"chunk width overflow: {rel[valid].max()}"
    segrel = np.where(valid, rel, -1).astype(np.float32)  # (B, S)

    # [b, t_within_ktile, ktile]
    segrelT = np.ascontiguousarray(
        segrel.reshape(BSZ, NKT, KT).transpose(0, 2, 1)
    )

    hidT = np.ascontiguousarray(hid.transpose(0, 2, 1))  # (B, D, S)
    emb_a = np.ascontiguousarray(
        E[:384].reshape(3, 128, NENT).transpose(1, 0, 2)
    )  # [128, 3, 32]
    emb_b = np.ascontiguousarray(E[384:400])  # [16, 32]
    prep = {"segrelT": segrelT, "base_c": base_c, "num_slots": num_slots}
    if mode == "bf16":
        prep["hidT"] = hidT.astype(NP_BF16)
        prep["emb_a"] = emb_a.astype(NP_BF16)
        prep["emb_b"] = emb_b.astype(NP_BF16)
        prep["ident"] = np.eye(NENT, dtype=np.float32).astype(NP_BF16)
    elif mode == "f32r":
        prep["hidT"] = hidT
        prep["emb_a"] = emb_a
        prep["emb_b"] = emb_b
        prep["ident"] = np.eye(NENT, dtype=np.float32)
    else:  # split
        hi = hidT.astype(NP_BF16)
        prep["hidT"] = hi
        prep["hidT_lo"] = (hidT - hi.astype(np.float32)).astype(NP_BF16)
        prep["emb_a"] = emb_a
        prep["emb_b"] = emb_b
        prep["ident"] = np.eye(NENT, dtype=np.float32)

    prep["iota"] = np.broadcast_to(
        np.arange(W, dtype=np.float32), (KT, W)
    ).copy()
    return prep


_PROG_CACHE = {}


def kernel(hidden_layers, binary_labels, entity_type_embs, _mode=None,
           _trace=False, _result_box=None):
    mode = _mode or MODE
    _patch_tile_tail_drain()
    _patch_compile_hook()
    prep = _host_prep(hidden_layers, binary_labels, entity_type_embs, mode)

    if mode not in _PROG_CACHE:
        _PROG_CACHE[mode] = _build_program(mode)
    nc = _PROG_CACHE[mode]

    in_maps = []
    for core in range(NCORES):
        sl = slice(core * BPC, (core + 1) * BPC)
        m = {
            "hidT": prep["hidT"][sl],
            "segrel": prep["segrelT"][sl],
            "iota": prep["iota"],
            "emb_a": prep["emb_a"],
            "emb_b": prep["emb_b"],
            "ident": prep["ident"],
        }
        if mode == "split":
            m["hidT_lo"] = prep["hidT_lo"][sl]
        in_maps.append(m)

    res = run_bass_kernel_spmd(
        nc, in_maps, core_ids=list(range(NCORES)), trace=_trace
    )
    if _result_box is not None:
        _result_box.append(res)

    logits = np.zeros((BSZ, SEQ, NENT), dtype=np.float32)
    base_c = prep["base_c"]
    for core in range(NCORES):
        co = res.results[core]["chunk_out"]  # (BPC, NCH, W, NENT)
        for bb in range(BPC):
            b = core * BPC + bb
            for c in range(NCH):
                base = int(base_c[b, c])
                logits[b, base : base + W] += co[bb, c]
    return logits, prep["num_slots"]


# revision 21
# speedup vs baseline: 2.1114x; 1.0411x over previous
"""Trainium2 Bass kernel for nn_EntityNamePredictor (BIO span extraction +
sum pooling + entity-type projection).

Strategy (pure data-parallel over batch, 8 samples per core):
  - Host: label preprocessing only (cumsum of B-tags -> per-token local slot
    ids relative to a fixed 256-token chunk grid), plus a transpose+cast of
    hidden_layers to [b, d, t] layout so the on-device projection matmul can
    contract over d with natural APs.
  - Device (per core, uniform SPMD program):
      proj^T[32, t] = E^T @ hiddenT          (PE, E stationary)
      proj[t, 32]   = PE transpose of proj^T
      onehot[t, l]  = (iota == seg_rel)      (DVE, exact 0/1 in bf16)
      chunk_logits[l, 32] += onehot^T @ proj (PE, per 256-token chunk)
    Each 256-token chunk emits a [128, 32] partial-logit block at a fixed
    DRAM address (slot placement is data-dependent, so it is resolved on
    host during unshard).
  - Host: overlap-add the per-chunk blocks into the final [B, S, 32] logits.

The segment-sum is exact: the one-hot matrix is 0/1 (exact in bf16) and all
accumulation happens in fp32 PSUM.  The only precision loss is the rounding
of hidden_layers / entity_type_embs entering the projection matmul:
  mode "bf16":  hidT+emb in bf16 (half DMA)       ~5e-3 rel
  mode "f32r":  hidT+emb in float32r              hw-dependent
  mode "split": hidT as hi+lo bf16, emb f32r      ~1e-5 rel
"""

import json

import numpy as np
import ml_dtypes

import concourse.bass as bass
import concourse.bass2jax as bass2jax
import concourse.bass_utils as bass_utils
import concourse.mybir as mybir
import concourse.tile as tile
from concourse.bass_utils import run_bass_kernel_spmd
from concourse.vector_clock import ScopedClock
import bass_rust

# ----------------------------------------------------------------------------
# Problem constants (hardcoded per the harness contract)
BSZ, SEQ, DIM, NENT = 64, 4096, 400, 32
NCORES = 8
BPC = BSZ // NCORES            # samples per core
KT = 128                       # tokens per k-tile
NKT = SEQ // KT                # k-tiles per sample (32)
CH = 256                       # tokens per chunk
NCH = SEQ // CH                # chunks per sample (16)
GRP = 512                      # tokens per projection group
NGRP = SEQ // GRP              # groups per sample (8)
W = 128                        # slot window per chunk (local slot ids 0..127)
NDSL = 4                       # d-slices of 100

BF16 = mybir.dt.bfloat16
F32 = mybir.dt.float32
F32R = mybir.dt.float32r
NP_BF16 = np.dtype(ml_dtypes.bfloat16)

# precision mode: "bf16", "f32r", "split"
MODE = "bf16"


def _patch_tile_tail_drain():
    """This walrus build rejects instructions with >2 sem waits; the Tile
    tail drain aggregates one wait per live semaphore.  Re-distribute the
    waits across single-wait SP no-ops emitted just before the drain."""
    if getattr(tile.TileContext, "_tail_drain_patched", False):
        return

    def _drain_and_barrier(self, tick_clock, wait_clock):
        nc = self.nc
        probe = nc.sync.nop(nofuse=True, hint="tail_drain_waits")
        wait_clock.add_sem_waits(
            probe.ins, ScopedClock({None: tick_clock.global_clock})
        )
        si = probe.ins.sync_info
        waits = list(si.on_wait) if si is not None else []
        upds = list(si.on_update) if si is not None else []
        probe.ins.sync_info = bass_rust.SyncInfo(on_wait=waits[:1], on_update=upds)
        for w in waits[1:]:
            n = nc.sync.nop(nofuse=True, hint="tail_drain_waits")
            n.ins.sync_info = bass_rust.SyncInfo(on_wait=[w], on_update=[])
        nc.sync.drain()
        nc.all_engine_barrier()
        assert self.sems is not None
        popped = nc._tile_sem_poison_stack.pop()
        assert popped is self._sem_poison
        nc.clear_and_free_semaphores(list(self.sems.allocated().values()))
        nc.all_engine_barrier()

    tile.TileContext._drain_and_barrier = _drain_and_barrier
    tile.TileContext._tail_drain_patched = True


# Max sync waits one instruction may carry through this walrus build.
_MAXW = 1


def _split_excess_waits(bir_json: bytes) -> bytes:
    """Walrus rejects instructions with too many sync-wait commands.  Move
    excess waits onto pure-wait EventSemaphore instructions inserted just
    before the offending instruction on the same engine (the sequencer
    executes its stream in order, so this is semantically identical)."""
    d = json.loads(bir_json)
    uid = 0
    changed = False
    for fn in d.get("functions", []):
        for blk in fn.get("blocks", []):
            out = []
            for inst in blk.get("instructions", []):
                si = inst.get("sync_info")
                ow = (si or {}).get("on_wait") or []
                if len(ow) > _MAXW:
                    changed = True
                    extra, keep = ow[:-_MAXW], ow[-_MAXW:]
                    for i in range(0, len(extra), _MAXW):
                        out.append({
                            "debug": 0,
                            "engine": inst["engine"],
                            "ins": [],
                            "outs": [],
                            "name": f"{inst['name']}-xw{uid}",
                            "opcode": "EventSemaphore",
                            "sync_info": {
                                "on_update": [],
                                "on_wait": extra[i : i + _MAXW],
                            },
                        })
                        uid += 1
                    si["on_wait"] = keep
                out.append(inst)
            blk["instructions"] = out
    if not changed:
        return bir_json
    return json.dumps(d).encode()


def _patch_compile_hook():
    if getattr(bass_utils, "_split_waits_patched", False):
        return
    orig = bass_utils.compile_bir_kernel

    def patched(bir_json, tmpdir, neff_name="file.neff"):
        return orig(_split_excess_waits(bir_json), tmpdir, neff_name=neff_name)

    bass_utils.compile_bir_kernel = patched
    if getattr(bass2jax, "compile_bir_kernel", None) is orig:
        bass2jax.compile_bir_kernel = patched
    bass_utils._split_waits_patched = True


def _build_program(mode: str, bufs_hid: int = 2, loop_k: int = 0,
                   only_dma: bool = False, no_dma: bool = False):
    """loop_k > 0 wraps the whole body in a For_i that repeats it loop_k
    times (benchmarking only).  only_dma / no_dma build stripped variants
    for bottleneck isolation (bench only)."""
    nc = bass.Bass("TRN2", target_bir_lowering=False, debug=False)

    hid_dt = {"bf16": BF16, "f32r": F32R, "split": BF16}[mode]
    emb_dt = {"bf16": BF16, "f32r": F32R, "split": F32R}[mode]
    prj_dt = BF16 if mode == "bf16" else F32  # projT sbuf / transpose dtype
    split_proj = mode in ("f32r", "split")    # hi/lo split of proj for mm_seg

    # DRAM parameters
    hidT = nc.dram_tensor("hidT", [BPC, DIM, SEQ], hid_dt, kind="ExternalInput")
    if mode == "split":
        hidT_lo = nc.dram_tensor(
            "hidT_lo", [BPC, DIM, SEQ], BF16, kind="ExternalInput"
        )
    segrel = nc.dram_tensor("segrel", [BPC, KT, NKT], F32, kind="ExternalInput")
    iota_in = nc.dram_tensor("iota", [KT, W], F32, kind="ExternalInput")
    emb_a_in = nc.dram_tensor("emb_a", [128, 3, NENT], emb_dt,
                              kind="ExternalInput")
    emb_b_in = nc.dram_tensor("emb_b", [16, NENT], emb_dt,
                              kind="ExternalInput")
    ident = nc.dram_tensor("ident", [NENT, NENT], prj_dt, kind="ExternalInput")
    out = nc.dram_tensor(
        "chunk_out", [BPC, NCH, W, NENT], F32, kind="ExternalOutput"
    )

    with tile.TileContext(nc) as tc:
        with (
            tc.tile_pool(name="const", bufs=1) as cpool,
            tc.tile_pool(name="hid", bufs=bufs_hid) as hpool,
            tc.tile_pool(name="proj", bufs=3) as ppool,
            tc.tile_pool(name="oneh", bufs=3) as opool,
            tc.tile_pool(name="outp", bufs=2) as outpool,
            tc.tile_pool(name="ps_projT", bufs=2, space="PSUM") as ps_projT,
            tc.tile_pool(name="ps_tp", bufs=2, space="PSUM") as ps_tp,
            tc.tile_pool(name="ps_log", bufs=3, space="PSUM") as ps_log,
        ):
            iota_t = cpool.tile([KT, W], F32)
            nc.sync.dma_start(iota_t[:], iota_in[:])
            emb_a = cpool.tile([128, 3, NENT], emb_dt)
            nc.sync.dma_start(emb_a[:], emb_a_in[:])
            emb_b = cpool.tile([16, NENT], emb_dt)
            nc.sync.dma_start(emb_b[:], emb_b_in[:])
            ident_t = cpool.tile([NENT, NENT], prj_dt)
            nc.sync.dma_start(ident_t[:], ident[:])
            # all samples' seg_rel columns in one load: [t, (b, k)]
            sr_t = cpool.tile([KT, BPC, NKT], F32)
            nc.sync.dma_start(
                sr_t[:], segrel.rearrange("b t k -> t b k")
            )

            import contextlib
            dma_engs = [nc.sync, nc.scalar]
            if no_dma:  # bench variant: preload one sample outside loop
                ht0 = cpool.tile([128, 3, SEQ], hid_dt)
                for j in range(3):
                    dma_engs[j % 2].dma_start(
                        ht0[:, j, :], hidT[0, 128 * j : 128 * (j + 1), :]
                    )
                ht0_b = cpool.tile([16, SEQ], hid_dt)
                nc.sync.dma_start(ht0_b[:], hidT[0, 384:400, :])
            loop_cm = tc.For_i(0, loop_k, 1) if loop_k else (
                contextlib.nullcontext()
            )
            with loop_cm:
              for b in range(BPC):
                # whole-sample loads, one DMA per 128-partition d-slice,
                # alternating the two HWDGE engines to spread queues
                if no_dma:
                    ht, ht_b = ht0, ht0_b
                else:
                    ht = hpool.tile([128, 3, SEQ], hid_dt, tag="hid")
                    for j in range(3):
                        dma_engs[j % 2].dma_start(
                            ht[:, j, :], hidT[b, 128 * j : 128 * (j + 1), :]
                        )
                    ht_b = hpool.tile([16, SEQ], hid_dt, tag="hidb")
                    dma_engs[1].dma_start(ht_b[:], hidT[b, 384:400, :])
                if only_dma:
                    continue
                if mode == "split":
                    ht_lo = hpool.tile([128, 3, SEQ], BF16, tag="hidlo")
                    for j in range(3):
                        dma_engs[j % 2].dma_start(
                            ht_lo[:, j, :],
                            hidT_lo[b, 128 * j : 128 * (j + 1), :],
                        )
                    ht_lo_b = hpool.tile([16, SEQ], BF16, tag="hidlob")
                    dma_engs[0].dma_start(ht_lo_b[:], hidT_lo[b, 384:400, :])
                # per-sample output accumulator in SBUF, one DMA at the end
                ob = outpool.tile([W, NCH, NENT], F32, tag="outsb")

                for g in range(NGRP):
                    # ---- projection: projT[32, 512] = E^T @ hiddenT group
                    pT = ps_projT.tile([NENT, GRP], F32, tag="pT")
                    n_pass = 2 if mode == "split" else 1
                    mm = 0
                    gsl = slice(g * GRP, (g + 1) * GRP)
                    for ip in range(n_pass):
                        src3 = ht if ip == 0 else ht_lo
                        src1 = ht_b if ip == 0 else ht_lo_b
                        for j in range(3):
                            nc.tensor.matmul(
                                pT[:],
                                emb_a[:, j, :],
                                src3[:, j, gsl],
                                start=(mm == 0),
                                stop=False,
                            )
                            mm += 1
                        nc.tensor.matmul(
                            pT[:],
                            emb_b[:],
                            src1[:, gsl],
                            start=False,
                            stop=(ip == n_pass - 1),
                        )
                        mm += 1

                    # cast / copy projT to SBUF in transpose dtype
                    pT_sb = ppool.tile([NENT, GRP], prj_dt, tag="pTsb")
                    nc.scalar.copy(pT_sb[:], pT[:])

                    # ---- transpose projT -> proj [128 tok, 32] x4 slices
                    tp = ps_tp.tile([KT, 4 * NENT], prj_dt, tag="tp")
                    for j in range(4):
                        nc.tensor.matmul(
                            tp[:, j * NENT : (j + 1) * NENT],
                            pT_sb[:, j * KT : (j + 1) * KT],
                            ident_t[:],
                            is_transpose=True,
                        )
                    proj_sb = ppool.tile([KT, 4 * NENT], BF16, tag="proj")
                    nc.scalar.copy(proj_sb[:], tp[:])
                    if split_proj:
                        proj_lo = ppool.tile([KT, 4 * NENT], BF16, tag="projlo")
                        nc.vector.tensor_sub(proj_lo[:], tp[:], proj_sb[:])

                    # ---- segment-sum: 2 chunks per group
                    for cc in range(2):
                        c = 2 * g + cc
                        pl = ps_log.tile([W, NENT], F32, tag="plog")
                        n_seg = 2 if split_proj else 1
                        mm2 = 0
                        for jj in range(2):  # k-tiles in chunk
                            k = 4 * g + 2 * cc + jj
                            oh = opool.tile([KT, W], BF16, tag="oneh")
                            nc.vector.tensor_scalar(
                                oh[:], iota_t[:], sr_t[:, b, k : k + 1], None,
                                mybir.AluOpType.is_equal,
                            )
                            sl = slice(
                                (2 * cc + jj) * NENT, (2 * cc + jj + 1) * NENT
                            )
                            for ip in range(n_seg):
                                src_p = proj_sb if ip == 0 else proj_lo
                                nc.tensor.matmul(
                                    pl[:],
                                    oh[:],
                                    src_p[:, sl],
                                    start=(mm2 == 0),
                                    stop=(mm2 == 2 * n_seg - 1),
                                )
                                mm2 += 1
                        nc.vector.tensor_copy(ob[:, c, :], pl[:])
                nc.sync.dma_start(
                    out[b].rearrange("c l e -> l c e"), ob[:]
                )
    return nc


def _host_prep(hidden_layers, binary_labels, entity_type_embs, mode):
    hid = np.asarray(hidden_layers, dtype=np.float32)
    lab = np.asarray(binary_labels)
    E = np.asarray(entity_type_embs, dtype=np.float32)

    is_b = (lab == 1)
    seg = np.cumsum(is_b.astype(np.int64), axis=1)  # (B, S) 1-based span count
    valid = ((lab == 1) | (lab == 2)) & (seg > 0)
    num_slots = seg[:, -1].astype(np.int32)

    # chunk grid: base slot of chunk c = open slot at token 256c
    base_c = np.maximum(
        np.concatenate(
            [np.zeros((BSZ, 1), np.int64), seg[:, CH - 1 : -1 : CH] - 1], axis=1
        ),
        0,
    )  # (B, NCH)
    slot = seg - 1  # 0-based slot id of each token (valid tokens only)
    rel = slot - np.repeat(base_c, CH, axis=1)
    if valid.any():
        assert rel[valid].min() >= 0
        assert rel[valid].max() < W, f"chunk width overflow: {rel[valid].max()}"
    segrel = np.where(valid, rel, -1).astype(np.float32)  # (B, S)

    # [b, t_within_ktile, ktile]
    segrelT = np.ascontiguousarray(
        segrel.reshape(BSZ, NKT, KT).transpose(0, 2, 1)
    )

    hidT = np.ascontiguousarray(hid.transpose(0, 2, 1))  # (B, D, S)
    emb_a = np.ascontiguousarray(
        E[:384].reshape(3, 128, NENT).transpose(1, 0, 2)
    )  # [128, 3, 32]
    emb_b = np.ascontiguousarray(E[384:400])  # [16, 32]
    prep = {"segrelT": segrelT, "base_c": base_c, "num_slots": num_slots}
    if mode == "bf16":
        prep["hidT"] = hidT.astype(NP_BF16)
        prep["emb_a"] = emb_a.astype(NP_BF16)
        prep["emb_b"] = emb_b.astype(NP_BF16)
        prep["ident"] = np.eye(NENT, dtype=np.float32).astype(NP_BF16)
    elif mode == "f32r":
        prep["hidT"] = hidT
        prep["emb_a"] = emb_a
        prep["emb_b"] = emb_b
        prep["ident"] = np.eye(NENT, dtype=np.float32)
    else:  # split
        hi = hidT.astype(NP_BF16)
        prep["hidT"] = hi
        prep["hidT_lo"] = (hidT - hi.astype(np.float32)).astype(NP_BF16)
        prep["emb_a"] = emb_a
        prep["emb_b"] = emb_b
        prep["ident"] = np.eye(NENT, dtype=np.float32)

    prep["iota"] = np.broadcast_to(
        np.arange(W, dtype=np.float32), (KT, W)
    ).copy()
    return prep


_PROG_CACHE = {}


def kernel(hidden_layers, binary_labels, entity_type_embs, _mode=None,
           _trace=False, _result_box=None):
    mode = _mode or MODE
    _patch_tile_tail_drain()
    _patch_compile_hook()
    prep = _host_prep(hidden_layers, binary_labels, entity_type_embs, mode)

    if mode not in _PROG_CACHE:
        _PROG_CACHE[mode] = _build_program(mode)
    nc = _PROG_CACHE[mode]

    in_maps = []
    for core in range(NCORES):
        sl = slice(core * BPC, (core + 1) * BPC)
        m = {
            "hidT": prep["hidT"][sl],
            "segrel": prep["segrelT"][sl],
            "iota": prep["iota"],
            "emb_a": prep["emb_a"],
            "emb_b": prep["emb_b"],
            "ident": prep["ident"],
        }
        if mode == "split":
            m["hidT_lo"] = prep["hidT_lo"][sl]
        in_maps.append(m)

    res = run_bass_kernel_spmd(
        nc, in_maps, core_ids=list(range(NCORES)), trace=_trace
    )
    if _result_box is not None:
        _result_box.append(res)

    logits = np.zeros((BSZ, SEQ, NENT), dtype=np.float32)
    base_c = prep["base_c"]
    for core in range(NCORES):
        co = res.results[core]["chunk_out"]  # (BPC, NCH, W, NENT)
        for bb in range(BPC):
            b = core * BPC + bb
            for c in range(NCH):
                base = int(base_c[b, c])
                logits[b, base : base + W] += co[bb, c]
    return logits, prep["num_slots"]


# revision 22
# speedup vs baseline: 2.4938x; 1.1811x over previous
"""Trainium2 Bass kernel for nn_EntityNamePredictor (BIO span extraction +
sum pooling + entity-type projection).

Strategy (pure data-parallel over batch, 8 samples per core):
  - Host: label preprocessing only (cumsum of B-tags -> per-token local slot
    ids relative to a fixed 256-token chunk grid), plus a transpose+cast of
    hidden_layers to [b, d, t] layout so the on-device projection matmul can
    contract over d with natural APs.
  - Device (per core, uniform SPMD program):
      proj^T[32, t] = E^T @ hiddenT          (PE, E stationary)
      proj[t, 32]   = PE transpose of proj^T
      onehot[t, l]  = (iota == seg_rel)      (DVE, exact 0/1 in bf16)
      chunk_logits[l, 32] += onehot^T @ proj (PE, per 256-token chunk)
    Each 256-token chunk emits a [128, 32] partial-logit block at a fixed
    DRAM address (slot placement is data-dependent, so it is resolved on
    host during unshard).
  - Host: overlap-add the per-chunk blocks into the final [B, S, 32] logits.

The segment-sum is exact: the one-hot matrix is 0/1 (exact in bf16) and all
accumulation happens in fp32 PSUM.  The only precision loss is the rounding
of hidden_layers / entity_type_embs entering the projection matmul:
  mode "bf16":  hidT+emb in bf16 (half DMA)       ~5e-3 rel
  mode "f32r":  hidT+emb in float32r              hw-dependent
  mode "split": hidT as hi+lo bf16, emb f32r      ~1e-5 rel
"""

import json

import numpy as np
import ml_dtypes

import concourse.bass as bass
import concourse.bass2jax as bass2jax
import concourse.bass_utils as bass_utils
import concourse.mybir as mybir
import concourse.tile as tile
from concourse.bass_utils import run_bass_kernel_spmd
from concourse.vector_clock import ScopedClock
import bass_rust

# ----------------------------------------------------------------------------
# Problem constants (hardcoded per the harness contract)
BSZ, SEQ, DIM, NENT = 64, 4096, 400, 32
NCORES = 8
BPC = BSZ // NCORES            # samples per core
KT = 128                       # tokens per k-tile
NKT = SEQ // KT                # k-tiles per sample (32)
CH = 256                       # tokens per chunk
NCH = SEQ // CH                # chunks per sample (16)
GRP = 512                      # tokens per projection group
NGRP = SEQ // GRP              # groups per sample (8)
W = 128                        # slot window per chunk (local slot ids 0..127)
NDSL = 4                       # d-slices of 100

BF16 = mybir.dt.bfloat16
F32 = mybir.dt.float32
F32R = mybir.dt.float32r
NP_BF16 = np.dtype(ml_dtypes.bfloat16)

# precision mode: "bf16", "f32r", "split"
MODE = "bf16"


def _patch_tile_tail_drain():
    """This walrus build rejects instructions with >2 sem waits; the Tile
    tail drain aggregates one wait per live semaphore.  Re-distribute the
    waits across single-wait SP no-ops emitted just before the drain."""
    if getattr(tile.TileContext, "_tail_drain_patched", False):
        return

    def _drain_and_barrier(self, tick_clock, wait_clock):
        nc = self.nc
        probe = nc.sync.nop(nofuse=True, hint="tail_drain_waits")
        wait_clock.add_sem_waits(
            probe.ins, ScopedClock({None: tick_clock.global_clock})
        )
        si = probe.ins.sync_info
        waits = list(si.on_wait) if si is not None else []
        upds = list(si.on_update) if si is not None else []
        probe.ins.sync_info = bass_rust.SyncInfo(on_wait=waits[:1], on_update=upds)
        for w in waits[1:]:
            n = nc.sync.nop(nofuse=True, hint="tail_drain_waits")
            n.ins.sync_info = bass_rust.SyncInfo(on_wait=[w], on_update=[])
        nc.sync.drain()
        nc.all_engine_barrier()
        assert self.sems is not None
        popped = nc._tile_sem_poison_stack.pop()
        assert popped is self._sem_poison
        nc.clear_and_free_semaphores(list(self.sems.allocated().values()))
        nc.all_engine_barrier()

    tile.TileContext._drain_and_barrier = _drain_and_barrier
    tile.TileContext._tail_drain_patched = True


# Max sync waits one instruction may carry through this walrus build.
_MAXW = 1


def _split_excess_waits(bir_json: bytes) -> bytes:
    """Walrus rejects instructions with too many sync-wait commands.  Move
    excess waits onto pure-wait EventSemaphore instructions inserted just
    before the offending instruction on the same engine (the sequencer
    executes its stream in order, so this is semantically identical)."""
    d = json.loads(bir_json)
    uid = 0
    changed = False
    for fn in d.get("functions", []):
        for blk in fn.get("blocks", []):
            out = []
            for inst in blk.get("instructions", []):
                si = inst.get("sync_info")
                ow = (si or {}).get("on_wait") or []
                if len(ow) > _MAXW:
                    changed = True
                    extra, keep = ow[:-_MAXW], ow[-_MAXW:]
                    for i in range(0, len(extra), _MAXW):
                        out.append({
                            "debug": 0,
                            "engine": inst["engine"],
                            "ins": [],
                            "outs": [],
                            "name": f"{inst['name']}-xw{uid}",
                            "opcode": "EventSemaphore",
                            "sync_info": {
                                "on_update": [],
                                "on_wait": extra[i : i + _MAXW],
                            },
                        })
                        uid += 1
                    si["on_wait"] = keep
                out.append(inst)
            blk["instructions"] = out
    if not changed:
        return bir_json
    return json.dumps(d).encode()


def _patch_compile_hook():
    if getattr(bass_utils, "_split_waits_patched", False):
        return
    orig = bass_utils.compile_bir_kernel

    def patched(bir_json, tmpdir, neff_name="file.neff"):
        return orig(_split_excess_waits(bir_json), tmpdir, neff_name=neff_name)

    bass_utils.compile_bir_kernel = patched
    if getattr(bass2jax, "compile_bir_kernel", None) is orig:
        bass2jax.compile_bir_kernel = patched
    bass_utils._split_waits_patched = True


def _build_program(mode: str, bufs_hid: int = 2, loop_k: int = 0,
                   only_dma: bool = False, no_dma: bool = False):
    """loop_k > 0 wraps the whole body in a For_i that repeats it loop_k
    times (benchmarking only).  only_dma / no_dma build stripped variants
    for bottleneck isolation (bench only)."""
    nc = bass.Bass("TRN2", target_bir_lowering=False, debug=False)

    hid_dt = {"bf16": BF16, "f32r": F32R, "split": BF16}[mode]
    emb_dt = {"bf16": BF16, "f32r": F32R, "split": F32R}[mode]
    prj_dt = BF16 if mode == "bf16" else F32  # projT sbuf / transpose dtype
    split_proj = mode in ("f32r", "split")    # hi/lo split of proj for mm_seg

    # DRAM parameters
    hidT = nc.dram_tensor("hidT", [BPC, DIM, SEQ], hid_dt, kind="ExternalInput")
    if mode == "split":
        hidT_lo = nc.dram_tensor(
            "hidT_lo", [BPC, DIM, SEQ], BF16, kind="ExternalInput"
        )
    segrel = nc.dram_tensor("segrel", [BPC, KT, NKT], F32, kind="ExternalInput")
    iota_in = nc.dram_tensor("iota", [KT, W], F32, kind="ExternalInput")
    emb_a_in = nc.dram_tensor("emb_a", [128, 3, NENT], emb_dt,
                              kind="ExternalInput")
    emb_b_in = nc.dram_tensor("emb_b", [16, NENT], emb_dt,
                              kind="ExternalInput")
    ident = nc.dram_tensor("ident", [NENT, NENT], prj_dt, kind="ExternalInput")
    out = nc.dram_tensor(
        "chunk_out", [BPC, NCH, W, NENT], F32, kind="ExternalOutput"
    )

    with tile.TileContext(nc) as tc:
        with (
            tc.tile_pool(name="const", bufs=1) as cpool,
            tc.tile_pool(name="hid", bufs=bufs_hid) as hpool,
            tc.tile_pool(name="proj", bufs=6) as ppool,
            tc.tile_pool(name="oneh", bufs=6) as opool,
            tc.tile_pool(name="outp", bufs=2) as outpool,
            tc.tile_pool(name="ps_projT", bufs=3, space="PSUM") as ps_projT,
            tc.tile_pool(name="ps_tp", bufs=2, space="PSUM") as ps_tp,
            tc.tile_pool(name="ps_log", bufs=3, space="PSUM") as ps_log,
        ):
            iota_t = cpool.tile([KT, W], F32)
            nc.sync.dma_start(iota_t[:], iota_in[:])
            emb_a = cpool.tile([128, 3, NENT], emb_dt)
            nc.sync.dma_start(emb_a[:], emb_a_in[:])
            emb_b = cpool.tile([16, NENT], emb_dt)
            nc.sync.dma_start(emb_b[:], emb_b_in[:])
            ident_t = cpool.tile([NENT, NENT], prj_dt)
            nc.sync.dma_start(ident_t[:], ident[:])
            # all samples' seg_rel columns in one load: [t, (b, k)]
            sr_t = cpool.tile([KT, BPC, NKT], F32)
            nc.sync.dma_start(
                sr_t[:], segrel.rearrange("b t k -> t b k")
            )

            import contextlib
            dma_engs = [nc.sync, nc.scalar]
            if no_dma:  # bench variant: preload one sample outside loop
                ht0 = cpool.tile([128, 3, SEQ], hid_dt)
                for j in range(3):
                    dma_engs[j % 2].dma_start(
                        ht0[:, j, :], hidT[0, 128 * j : 128 * (j + 1), :]
                    )
                ht0_b = cpool.tile([16, SEQ], hid_dt)
                nc.sync.dma_start(ht0_b[:], hidT[0, 384:400, :])
            loop_cm = tc.For_i(0, loop_k, 1) if loop_k else (
                contextlib.nullcontext()
            )
            with loop_cm:
              for b in range(BPC):
                # whole-sample loads, one DMA per 128-partition d-slice,
                # alternating the two HWDGE engines to spread queues
                if no_dma:
                    ht, ht_b = ht0, ht0_b
                else:
                    ht = hpool.tile([128, 3, SEQ], hid_dt, tag="hid")
                    for j in range(3):
                        dma_engs[j % 2].dma_start(
                            ht[:, j, :], hidT[b, 128 * j : 128 * (j + 1), :]
                        )
                    ht_b = hpool.tile([16, SEQ], hid_dt, tag="hidb")
                    dma_engs[1].dma_start(ht_b[:], hidT[b, 384:400, :])
                if only_dma:
                    continue
                if mode == "split":
                    ht_lo = hpool.tile([128, 3, SEQ], BF16, tag="hidlo")
                    for j in range(3):
                        dma_engs[j % 2].dma_start(
                            ht_lo[:, j, :],
                            hidT_lo[b, 128 * j : 128 * (j + 1), :],
                        )
                    ht_lo_b = hpool.tile([16, SEQ], BF16, tag="hidlob")
                    dma_engs[0].dma_start(ht_lo_b[:], hidT_lo[b, 384:400, :])
                # per-sample output accumulator in SBUF, one DMA at the end
                ob = outpool.tile([W, NCH, NENT], F32, tag="outsb")

                for g in range(NGRP):
                    # ---- projection: projT[32, 512] = E^T @ hiddenT group
                    pT = ps_projT.tile([NENT, GRP], F32, tag="pT")
                    n_pass = 2 if mode == "split" else 1
                    mm = 0
                    gsl = slice(g * GRP, (g + 1) * GRP)
                    for ip in range(n_pass):
                        src3 = ht if ip == 0 else ht_lo
                        src1 = ht_b if ip == 0 else ht_lo_b
                        for j in range(3):
                            nc.tensor.matmul(
                                pT[:],
                                emb_a[:, j, :],
                                src3[:, j, gsl],
                                start=(mm == 0),
                                stop=False,
                            )
                            mm += 1
                        nc.tensor.matmul(
                            pT[:],
                            emb_b[:],
                            src1[:, gsl],
                            start=False,
                            stop=(ip == n_pass - 1),
                        )
                        mm += 1

                    # cast / copy projT to SBUF in transpose dtype
                    pT_sb = ppool.tile([NENT, GRP], prj_dt, tag="pTsb")
                    nc.scalar.copy(pT_sb[:], pT[:])

                    # ---- transpose projT -> proj [128 tok, 32] x4 slices
                    tp = ps_tp.tile([KT, 4 * NENT], prj_dt, tag="tp")
                    for j in range(4):
                        nc.tensor.matmul(
                            tp[:, j * NENT : (j + 1) * NENT],
                            pT_sb[:, j * KT : (j + 1) * KT],
                            ident_t[:],
                            is_transpose=True,
                        )
                    proj_sb = ppool.tile([KT, 4 * NENT], BF16, tag="proj")
                    nc.scalar.copy(proj_sb[:], tp[:])
                    if split_proj:
                        proj_lo = ppool.tile([KT, 4 * NENT], BF16, tag="projlo")
                        nc.vector.tensor_sub(proj_lo[:], tp[:], proj_sb[:])

                    # ---- segment-sum: 2 chunks per group
                    for cc in range(2):
                        c = 2 * g + cc
                        pl = ps_log.tile([W, NENT], F32, tag="plog")
                        n_seg = 2 if split_proj else 1
                        mm2 = 0
                        for jj in range(2):  # k-tiles in chunk
                            k = 4 * g + 2 * cc + jj
                            oh = opool.tile([KT, W], BF16, tag="oneh")
                            nc.vector.tensor_scalar(
                                oh[:], iota_t[:], sr_t[:, b, k : k + 1], None,
                                mybir.AluOpType.is_equal,
                            )
                            sl = slice(
                                (2 * cc + jj) * NENT, (2 * cc + jj + 1) * NENT
                            )
                            for ip in range(n_seg):
                                src_p = proj_sb if ip == 0 else proj_lo
                                nc.tensor.matmul(
                                    pl[:],
                                    oh[:],
                                    src_p[:, sl],
                                    start=(mm2 == 0),
                                    stop=(mm2 == 2 * n_seg - 1),
                                )
                                mm2 += 1
                        nc.vector.tensor_copy(ob[:, c, :], pl[:])
                nc.sync.dma_start(
                    out[b].rearrange("c l e -> l c e"), ob[:]
                )
    return nc


def _host_prep(hidden_layers, binary_labels, entity_type_embs, mode):
    hid = np.asarray(hidden_layers, dtype=np.float32)
    lab = np.asarray(binary_labels)
    E = np.asarray(entity_type_embs, dtype=np.float32)

    is_b = (lab == 1)
    seg = np.cumsum(is_b.astype(np.int64), axis=1)  # (B, S) 1-based span count
    valid = ((lab == 1) | (lab == 2)) & (seg > 0)
    num_slots = seg[:, -1].astype(np.int32)

    # chunk grid: base slot of chunk c = open slot at token 256c
    base_c = np.maximum(
        np.concatenate(
            [np.zeros((BSZ, 1), np.int64), seg[:, CH - 1 : -1 : CH] - 1], axis=1
        ),
        0,
    )  # (B, NCH)
    slot = seg - 1  # 0-based slot id of each token (valid tokens only)
    rel = slot - np.repeat(base_c, CH, axis=1)
    if valid.any():
        assert rel[valid].min() >= 0
        assert rel[valid].max() < W, f"chunk width overflow: {rel[valid].max()}"
    segrel = np.where(valid, rel, -1).astype(np.float32)  # (B, S)

    # [b, t_within_ktile, ktile]
    segrelT = np.ascontiguousarray(
        segrel.reshape(BSZ, NKT, KT).transpose(0, 2, 1)
    )

    hidT = np.ascontiguousarray(hid.transpose(0, 2, 1))  # (B, D, S)
    emb_a = np.ascontiguousarray(
        E[:384].reshape(3, 128, NENT).transpose(1, 0, 2)
    )  # [128, 3, 32]
    emb_b = np.ascontiguousarray(E[384:400])  # [16, 32]
    prep = {"segrelT": segrelT, "base_c": base_c, "num_slots": num_slots}
    if mode == "bf16":
        prep["hidT"] = hidT.astype(NP_BF16)
        prep["emb_a"] = emb_a.astype(NP_BF16)
        prep["emb_b"] = emb_b.astype(NP_BF16)
        prep["ident"] = np.eye(NENT, dtype=np.float32).astype(NP_BF16)
    elif mode == "f32r":
        prep["hidT"] = hidT
        prep["emb_a"] = emb_a
        prep["emb_b"] = emb_b
        prep["ident"] = np.eye(NENT, dtype=np.float32)
    else:  # split
        hi = hidT.astype(NP_BF16)
        prep["hidT"] = hi
        prep["hidT_lo"] = (hidT - hi.astype(np.float32)).astype(NP_BF16)
        prep["emb_a"] = emb_a
        prep["emb_b"] = emb_b
        prep["ident"] = np.eye(NENT, dtype=np.float32)

    prep["iota"] = np.broadcast_to(
        np.arange(W, dtype=np.float32), (KT, W)
    ).copy()
    return prep


_PROG_CACHE = {}


def kernel(hidden_layers, binary_labels, entity_type_embs, _mode=None,
           _trace=False, _result_box=None):
    mode = _mode or MODE
    _patch_tile_tail_drain()
    _patch_compile_hook()
    prep = _host_prep(hidden_layers, binary_labels, entity_type_embs, mode)

    if mode not in _PROG_CACHE:
        _PROG_CACHE[mode] = _build_program(mode)
    nc = _PROG_CACHE[mode]

    in_maps = []
    for core in range(NCORES):
        sl = slice(core * BPC, (core + 1) * BPC)
        m = {
            "hidT": prep["hidT"][sl],
            "segrel": prep["segrelT"][sl],
            "iota": prep["iota"],
            "emb_a": prep["emb_a"],
            "emb_b": prep["emb_b"],
            "ident": prep["ident"],
        }
        if mode == "split":
            m["hidT_lo"] = prep["hidT_lo"][sl]
        in_maps.append(m)

    res = run_bass_kernel_spmd(
        nc, in_maps, core_ids=list(range(NCORES)), trace=_trace
    )
    if _result_box is not None:
        _result_box.append(res)

    logits = np.zeros((BSZ, SEQ, NENT), dtype=np.float32)
    base_c = prep["base_c"]
    for core in range(NCORES):
        co = res.results[core]["chunk_out"]  # (BPC, NCH, W, NENT)
        for bb in range(BPC):
            b = core * BPC + bb
            for c in range(NCH):
                base = int(base_c[b, c])
                logits[b, base : base + W] += co[bb, c]
    return logits, prep["num_slots"]


# revision 24
# speedup vs baseline: 2.8213x; 1.1313x over previous
"""Trainium2 Bass kernel for nn_EntityNamePredictor (BIO span extraction +
sum pooling + entity-type projection).

Strategy (pure data-parallel over batch, 8 samples per core):
  - Host: label preprocessing only (cumsum of B-tags -> per-token local slot
    ids relative to a fixed 256-token chunk grid), plus a transpose+cast of
    hidden_layers to [b, d, t] layout so the on-device projection matmul can
    contract over d with natural APs.
  - Device (per core, uniform SPMD program):
      proj^T[32, t] = E^T @ hiddenT          (PE, E stationary)
      proj[t, 32]   = PE transpose of proj^T
      onehot[t, l]  = (iota == seg_rel)      (DVE, exact 0/1 in bf16)
      chunk_logits[l, 32] += onehot^T @ proj (PE, per 256-token chunk)
    Each 256-token chunk emits a [128, 32] partial-logit block at a fixed
    DRAM address (slot placement is data-dependent, so it is resolved on
    host during unshard).
  - Host: overlap-add the per-chunk blocks into the final [B, S, 32] logits.

The segment-sum is exact: the one-hot matrix is 0/1 (exact in bf16) and all
accumulation happens in fp32 PSUM.  The only precision loss is the rounding
of hidden_layers / entity_type_embs entering the projection matmul:
  mode "bf16":  hidT+emb in bf16 (half DMA)       ~5e-3 rel
  mode "f32r":  hidT+emb in float32r              hw-dependent
  mode "split": hidT as hi+lo bf16, emb f32r      ~1e-5 rel
"""

import json

import numpy as np
import ml_dtypes

import concourse.bass as bass
import concourse.bass2jax as bass2jax
import concourse.bass_utils as bass_utils
import concourse.mybir as mybir
import concourse.tile as tile
from concourse.bass_utils import run_bass_kernel_spmd
from concourse.vector_clock import ScopedClock
import bass_rust

# ----------------------------------------------------------------------------
# Problem constants (hardcoded per the harness contract)
BSZ, SEQ, DIM, NENT = 64, 4096, 400, 32
NCORES = 8
BPC = BSZ // NCORES            # samples per core
KT = 128                       # tokens per k-tile
NKT = SEQ // KT                # k-tiles per sample (32)
CH = 256                       # tokens per chunk
NCH = SEQ // CH                # chunks per sample (16)
GRP = 512                      # tokens per projection group
NGRP = SEQ // GRP              # groups per sample (8)
W = 128                        # slot window per chunk (local slot ids 0..127)
NDSL = 4                       # d-slices of 100

BF16 = mybir.dt.bfloat16
F32 = mybir.dt.float32
F32R = mybir.dt.float32r
NP_BF16 = np.dtype(ml_dtypes.bfloat16)

# precision mode: "bf16", "f32r", "split"
MODE = "bf16"


def _patch_tile_tail_drain():
    """This walrus build rejects instructions with >2 sem waits; the Tile
    tail drain aggregates one wait per live semaphore.  Re-distribute the
    waits across single-wait SP no-ops emitted just before the drain."""
    if getattr(tile.TileContext, "_tail_drain_patched", False):
        return

    def _drain_and_barrier(self, tick_clock, wait_clock):
        nc = self.nc
        probe = nc.sync.nop(nofuse=True, hint="tail_drain_waits")
        wait_clock.add_sem_waits(
            probe.ins, ScopedClock({None: tick_clock.global_clock})
        )
        si = probe.ins.sync_info
        waits = list(si.on_wait) if si is not None else []
        upds = list(si.on_update) if si is not None else []
        probe.ins.sync_info = bass_rust.SyncInfo(on_wait=waits[:1], on_update=upds)
        for w in waits[1:]:
            n = nc.sync.nop(nofuse=True, hint="tail_drain_waits")
            n.ins.sync_info = bass_rust.SyncInfo(on_wait=[w], on_update=[])
        nc.sync.drain()
        nc.all_engine_barrier()
        assert self.sems is not None
        popped = nc._tile_sem_poison_stack.pop()
        assert popped is self._sem_poison
        nc.clear_and_free_semaphores(list(self.sems.allocated().values()))
        nc.all_engine_barrier()

    tile.TileContext._drain_and_barrier = _drain_and_barrier
    tile.TileContext._tail_drain_patched = True


# Max sync waits one instruction may carry through this walrus build.
_MAXW = 1


def _split_excess_waits(bir_json: bytes) -> bytes:
    """Walrus rejects instructions with too many sync-wait commands.  Move
    excess waits onto pure-wait EventSemaphore instructions inserted just
    before the offending instruction on the same engine (the sequencer
    executes its stream in order, so this is semantically identical)."""
    d = json.loads(bir_json)
    uid = 0
    changed = False
    for fn in d.get("functions", []):
        for blk in fn.get("blocks", []):
            out = []
            for inst in blk.get("instructions", []):
                si = inst.get("sync_info")
                ow = (si or {}).get("on_wait") or []
                if len(ow) > _MAXW:
                    changed = True
                    extra, keep = ow[:-_MAXW], ow[-_MAXW:]
                    for i in range(0, len(extra), _MAXW):
                        out.append({
                            "debug": 0,
                            "engine": inst["engine"],
                            "ins": [],
                            "outs": [],
                            "name": f"{inst['name']}-xw{uid}",
                            "opcode": "EventSemaphore",
                            "sync_info": {
                                "on_update": [],
                                "on_wait": extra[i : i + _MAXW],
                            },
                        })
                        uid += 1
                    si["on_wait"] = keep
                out.append(inst)
            blk["instructions"] = out
    if not changed:
        return bir_json
    return json.dumps(d).encode()


def _patch_compile_hook():
    if getattr(bass_utils, "_split_waits_patched", False):
        return
    orig = bass_utils.compile_bir_kernel

    def patched(bir_json, tmpdir, neff_name="file.neff"):
        return orig(_split_excess_waits(bir_json), tmpdir, neff_name=neff_name)

    bass_utils.compile_bir_kernel = patched
    if getattr(bass2jax, "compile_bir_kernel", None) is orig:
        bass2jax.compile_bir_kernel = patched
    bass_utils._split_waits_patched = True


def _build_program(mode: str, bufs_hid: int = 2, loop_k: int = 0,
                   only_dma: bool = False, no_dma: bool = False,
                   direct: bool = True):
    """loop_k > 0 wraps the whole body in a For_i that repeats it loop_k
    times (benchmarking only).  only_dma / no_dma build stripped variants
    for bottleneck isolation (bench only).  direct=True computes proj per
    k-tile with hidden as the matmul stationary (no PE transposes)."""
    nc = bass.Bass("TRN2", target_bir_lowering=False, debug=False)

    hid_dt = {"bf16": BF16, "f32r": F32R, "split": BF16}[mode]
    emb_dt = {"bf16": BF16, "f32r": F32R, "split": F32R}[mode]
    prj_dt = BF16 if mode == "bf16" else F32  # projT sbuf / transpose dtype
    split_proj = mode in ("f32r", "split")    # hi/lo split of proj for mm_seg

    # DRAM parameters
    hidT = nc.dram_tensor("hidT", [BPC, DIM, SEQ], hid_dt, kind="ExternalInput")
    if mode == "split":
        hidT_lo = nc.dram_tensor(
            "hidT_lo", [BPC, DIM, SEQ], BF16, kind="ExternalInput"
        )
    segrel = nc.dram_tensor("segrel", [BPC, KT, NKT], F32, kind="ExternalInput")
    iota_in = nc.dram_tensor("iota", [KT, W], F32, kind="ExternalInput")
    emb_a_in = nc.dram_tensor("emb_a", [128, 3, NENT], emb_dt,
                              kind="ExternalInput")
    emb_b_in = nc.dram_tensor("emb_b", [16, NENT], emb_dt,
                              kind="ExternalInput")
    ident = nc.dram_tensor("ident", [NENT, NENT], prj_dt, kind="ExternalInput")
    out = nc.dram_tensor(
        "chunk_out", [BPC, NCH, W, NENT], F32, kind="ExternalOutput"
    )

    with tile.TileContext(nc) as tc:
        with (
            tc.tile_pool(name="const", bufs=1) as cpool,
            tc.tile_pool(name="hid", bufs=bufs_hid) as hpool,
            tc.tile_pool(name="proj", bufs=6) as ppool,
            tc.tile_pool(name="oneh", bufs=6) as opool,
            tc.tile_pool(name="outp", bufs=2) as outpool,
            tc.tile_pool(name="ps_projT", bufs=3, space="PSUM") as ps_projT,
            tc.tile_pool(name="ps_tp", bufs=2, space="PSUM") as ps_tp,
            tc.tile_pool(name="ps_log", bufs=3, space="PSUM") as ps_log,
        ):
            iota_t = cpool.tile([KT, W], F32)
            nc.sync.dma_start(iota_t[:], iota_in[:])
            emb_a = cpool.tile([128, 3, NENT], emb_dt)
            nc.sync.dma_start(emb_a[:], emb_a_in[:])
            emb_b = cpool.tile([16, NENT], emb_dt)
            nc.sync.dma_start(emb_b[:], emb_b_in[:])
            ident_t = cpool.tile([NENT, NENT], prj_dt)
            nc.sync.dma_start(ident_t[:], ident[:])
            # all samples' seg_rel columns in one load: [t, (b, k)]
            sr_t = cpool.tile([KT, BPC, NKT], F32)
            nc.sync.dma_start(
                sr_t[:], segrel.rearrange("b t k -> t b k")
            )

            import contextlib
            dma_engs = [nc.sync, nc.scalar]
            if no_dma:  # bench variant: preload one sample outside loop
                ht0 = cpool.tile([128, 3, SEQ], hid_dt)
                for j in range(3):
                    dma_engs[j % 2].dma_start(
                        ht0[:, j, :], hidT[0, 128 * j : 128 * (j + 1), :]
                    )
                ht0_b = cpool.tile([16, SEQ], hid_dt)
                nc.sync.dma_start(ht0_b[:], hidT[0, 384:400, :])
            loop_cm = tc.For_i(0, loop_k, 1) if loop_k else (
                contextlib.nullcontext()
            )
            with loop_cm:
              for b in range(BPC):
                # whole-sample loads, one DMA per 128-partition d-slice,
                # alternating the two HWDGE engines to spread queues
                if no_dma:
                    ht, ht_b = ht0, ht0_b
                else:
                    ht = hpool.tile([128, 3, SEQ], hid_dt, tag="hid")
                    for j in range(3):
                        dma_engs[j % 2].dma_start(
                            ht[:, j, :], hidT[b, 128 * j : 128 * (j + 1), :]
                        )
                    ht_b = hpool.tile([16, SEQ], hid_dt, tag="hidb")
                    dma_engs[1].dma_start(ht_b[:], hidT[b, 384:400, :])
                if only_dma:
                    continue
                if direct:
                    ob = outpool.tile([W, NCH, NENT], F32, tag="outsb")
                    for c in range(NCH):
                        pl = ps_log.tile([W, NENT], F32, tag="plog")
                        for jj in range(2):
                            k = 2 * c + jj
                            ksl = slice(k * KT, (k + 1) * KT)
                            pp = ps_projT.tile([KT, NENT], F32, tag="pp")
                            n_pass = 2 if mode == "split" else 1
                            mm = 0
                            for ip in range(n_pass):
                                s3 = ht if ip == 0 else ht_lo
                                s1 = ht_b if ip == 0 else ht_lo_b
                                for j in range(3):
                                    nc.tensor.matmul(
                                        pp[:], s3[:, j, ksl], emb_a[:, j, :],
                                        start=(mm == 0), stop=False,
                                    )
                                    mm += 1
                                nc.tensor.matmul(
                                    pp[:], s1[:, ksl], emb_b[:],
                                    start=False, stop=(ip == n_pass - 1),
                                )
                                mm += 1
                            pj = ppool.tile([KT, NENT], BF16, tag="proj")
                            nc.scalar.copy(pj[:], pp[:])
                            if split_proj:
                                pj_lo = ppool.tile([KT, NENT], BF16,
                                                   tag="projlo")
                                nc.vector.tensor_sub(pj_lo[:], pp[:], pj[:])
                            oh = opool.tile([KT, W], BF16, tag="oneh")
                            nc.vector.tensor_scalar(
                                oh[:], iota_t[:], sr_t[:, b, k : k + 1], None,
                                mybir.AluOpType.is_equal,
                            )
                            n_seg = 2 if split_proj else 1
                            for ip in range(n_seg):
                                nc.tensor.matmul(
                                    pl[:], oh[:],
                                    pj[:] if ip == 0 else pj_lo[:],
                                    start=(jj == 0 and ip == 0),
                                    stop=(jj == 1 and ip == n_seg - 1),
                                )
                        nc.vector.tensor_copy(ob[:, c, :], pl[:])
                    nc.sync.dma_start(
                        out[b].rearrange("c l e -> l c e"), ob[:]
                    )
                    continue
                if mode == "split":
                    ht_lo = hpool.tile([128, 3, SEQ], BF16, tag="hidlo")
                    for j in range(3):
                        dma_engs[j % 2].dma_start(
                            ht_lo[:, j, :],
                            hidT_lo[b, 128 * j : 128 * (j + 1), :],
                        )
                    ht_lo_b = hpool.tile([16, SEQ], BF16, tag="hidlob")
                    dma_engs[0].dma_start(ht_lo_b[:], hidT_lo[b, 384:400, :])
                # per-sample output accumulator in SBUF, one DMA at the end
                ob = outpool.tile([W, NCH, NENT], F32, tag="outsb")

                for g in range(NGRP):
                    # ---- projection: projT[32, 512] = E^T @ hiddenT group
                    pT = ps_projT.tile([NENT, GRP], F32, tag="pT")
                    n_pass = 2 if mode == "split" else 1
                    mm = 0
                    gsl = slice(g * GRP, (g + 1) * GRP)
                    for ip in range(n_pass):
                        src3 = ht if ip == 0 else ht_lo
                        src1 = ht_b if ip == 0 else ht_lo_b
                        for j in range(3):
                            nc.tensor.matmul(
                                pT[:],
                                emb_a[:, j, :],
                                src3[:, j, gsl],
                                start=(mm == 0),
                                stop=False,
                            )
                            mm += 1
                        nc.tensor.matmul(
                            pT[:],
                            emb_b[:],
                            src1[:, gsl],
                            start=False,
                            stop=(ip == n_pass - 1),
                        )
                        mm += 1

                    # cast / copy projT to SBUF in transpose dtype
                    pT_sb = ppool.tile([NENT, GRP], prj_dt, tag="pTsb")
                    nc.scalar.copy(pT_sb[:], pT[:])

                    # ---- transpose projT -> proj [128 tok, 32] x4 slices
                    tp = ps_tp.tile([KT, 4 * NENT], prj_dt, tag="tp")
                    for j in range(4):
                        nc.tensor.matmul(
                            tp[:, j * NENT : (j + 1) * NENT],
                            pT_sb[:, j * KT : (j + 1) * KT],
                            ident_t[:],
                            is_transpose=True,
                        )
                    proj_sb = ppool.tile([KT, 4 * NENT], BF16, tag="proj")
                    nc.scalar.copy(proj_sb[:], tp[:])
                    if split_proj:
                        proj_lo = ppool.tile([KT, 4 * NENT], BF16, tag="projlo")
                        nc.vector.tensor_sub(proj_lo[:], tp[:], proj_sb[:])

                    # ---- segment-sum: 2 chunks per group
                    for cc in range(2):
                        c = 2 * g + cc
                        pl = ps_log.tile([W, NENT], F32, tag="plog")
                        n_seg = 2 if split_proj else 1
                        mm2 = 0
                        for jj in range(2):  # k-tiles in chunk
                            k = 4 * g + 2 * cc + jj
                            oh = opool.tile([KT, W], BF16, tag="oneh")
                            nc.vector.tensor_scalar(
                                oh[:], iota_t[:], sr_t[:, b, k : k + 1], None,
                                mybir.AluOpType.is_equal,
                            )
                            sl = slice(
                                (2 * cc + jj) * NENT, (2 * cc + jj + 1) * NENT
                            )
                            for ip in range(n_seg):
                                src_p = proj_sb if ip == 0 else proj_lo
                                nc.tensor.matmul(
                                    pl[:],
                                    oh[:],
                                    src_p[:, sl],
                                    start=(mm2 == 0),
                                    stop=(mm2 == 2 * n_seg - 1),
                                )
                                mm2 += 1
                        nc.vector.tensor_copy(ob[:, c, :], pl[:])
                nc.sync.dma_start(
                    out[b].rearrange("c l e -> l c e"), ob[:]
                )
    return nc


def _host_prep(hidden_layers, binary_labels, entity_type_embs, mode):
    hid = np.asarray(hidden_layers, dtype=np.float32)
    lab = np.asarray(binary_labels)
    E = np.asarray(entity_type_embs, dtype=np.float32)

    is_b = (lab == 1)
    seg = np.cumsum(is_b.astype(np.int64), axis=1)  # (B, S) 1-based span count
    valid = ((lab == 1) | (lab == 2)) & (seg > 0)
    num_slots = seg[:, -1].astype(np.int32)

    # chunk grid: base slot of chunk c = open slot at token 256c
    base_c = np.maximum(
        np.concatenate(
            [np.zeros((BSZ, 1), np.int64), seg[:, CH - 1 : -1 : CH] - 1], axis=1
        ),
        0,
    )  # (B, NCH)
    slot = seg - 1  # 0-based slot id of each token (valid tokens only)
    rel = slot - np.repeat(base_c, CH, axis=1)
    if valid.any():
        assert rel[valid].min() >= 0
        assert rel[valid].max() < W, f"chunk width overflow: {rel[valid].max()}"
    segrel = np.where(valid, rel, -1).astype(np.float32)  # (B, S)

    # [b, t_within_ktile, ktile]
    segrelT = np.ascontiguousarray(
        segrel.reshape(BSZ, NKT, KT).transpose(0, 2, 1)
    )

    hidT = np.ascontiguousarray(hid.transpose(0, 2, 1))  # (B, D, S)
    emb_a = np.ascontiguousarray(
        E[:384].reshape(3, 128, NENT).transpose(1, 0, 2)
    )  # [128, 3, 32]
    emb_b = np.ascontiguousarray(E[384:400])  # [16, 32]
    prep = {"segrelT": segrelT, "base_c": base_c, "num_slots": num_slots}
    if mode == "bf16":
        prep["hidT"] = hidT.astype(NP_BF16)
        prep["emb_a"] = emb_a.astype(NP_BF16)
        prep["emb_b"] = emb_b.astype(NP_BF16)
        prep["ident"] = np.eye(NENT, dtype=np.float32).astype(NP_BF16)
    elif mode == "f32r":
        prep["hidT"] = hidT
        prep["emb_a"] = emb_a
        prep["emb_b"] = emb_b
        prep["ident"] = np.eye(NENT, dtype=np.float32)
    else:  # split
        hi = hidT.astype(NP_BF16)
        prep["hidT"] = hi
        prep["hidT_lo"] = (hidT - hi.astype(np.float32)).astype(NP_BF16)
        prep["emb_a"] = emb_a
        prep["emb_b"] = emb_b
        prep["ident"] = np.eye(NENT, dtype=np.float32)

    prep["iota"] = np.broadcast_to(
        np.arange(W, dtype=np.float32), (KT, W)
    ).copy()
    return prep


_PROG_CACHE = {}


def kernel(hidden_layers, binary_labels, entity_type_embs, _mode=None,
           _trace=False, _result_box=None):
    mode = _mode or MODE
    _patch_tile_tail_drain()
    _patch_compile_hook()
    prep = _host_prep(hidden_layers, binary_labels, entity_type_embs, mode)

    if mode not in _PROG_CACHE:
        _PROG_CACHE[mode] = _build_program(mode)
    nc = _PROG_CACHE[mode]

    in_maps = []
    for core in range(NCORES):
        sl = slice(core * BPC, (core + 1) * BPC)
        m = {
            "hidT": prep["hidT"][sl],
            "segrel": prep["segrelT"][sl],
            "iota": prep["iota"],
            "emb_a": prep["emb_a"],
            "emb_b": prep["emb_b"],
            "ident": prep["ident"],
        }
        if mode == "split":
            m["hidT_lo"] = prep["hidT_lo"][sl]
        in_maps.append(m)

    res = run_bass_kernel_spmd(
        nc, in_maps, core_ids=list(range(NCORES)), trace=_trace
    )
    if _result_box is not None:
        _result_box.append(res)

    logits = np.zeros((BSZ, SEQ, NENT), dtype=np.float32)
    base_c = prep["base_c"]
    for core in range(NCORES):
        co = res.results[core]["chunk_out"]  # (BPC, NCH, W, NENT)
        for bb in range(BPC):
            b = core * BPC + bb
            for c in range(NCH):
                base = int(base_c[b, c])
                logits[b, base : base + W] += co[bb, c]
    return logits, prep["num_slots"]
